# revision 17
# baseline (speedup 1.0000x reference)
"""DocRE model kernel for 8 Trainium2 NeuronCores (Bass/Tile).

Core (b, i): b = batch; i indexes the attention c-half and the pair-half.
Stage 1: indirect-DMA gathers of att/seq mention rows; weight-matmul pooling
into ent_att / logsumexp ent_emb.  Stage 2: one-hot matmul pair gathers,
fused product + head-reduce -> ht_att, pair AllReduce of rs/norm partials.
Stage 3: FFN GEMMs (bf16) with the attention normalization folded in as a
per-partition scale; block-bilinear generated on DVE with broadcast APs,
transposed by the DMA xbar, GEMM vs streamed bil_W -> logits.  PU risk via
ones-matmul partials + 8-way AllReduce.
"""

import sys
from contextlib import ExitStack

import numpy as np

sys.path.insert(0, "/opt/trn_rl_repo")

import concourse.bass as bass  # noqa: E402
import concourse.tile as tile  # noqa: E402
from concourse import bacc, mybir  # noqa: E402

FP = mybir.dt.float32
BF = mybir.dt.bfloat16
I32 = mybir.dt.int32
AF = mybir.ActivationFunctionType
ALU = mybir.AluOpType
AX = mybir.AxisListType

B, H, C, D = 4, 12, 1024, 768
E, M, P = 30, 8, 600
L, RELS = 97, 96
EMB, BLK = 768, 64
NK = EMB // BLK
CH = C // 2
EM = E * M
NPC = 300
NPAD = 384
NT = 3
OFFSET = 1
EPS = 1.2e-4  # 1e-5 * H folded into the un-meaned sums
NCORES = 8
RSROWS = 776  # 768 rs rows + nsum row + pad to even

LAST_EXEC_NS = None
_PAIR_GROUPS = [[0, 1], [2, 3], [4, 5], [6, 7]]
_ALL_GROUPS = [list(range(NCORES))]


def build_program(reps=1, no_collectives=False, num_devices=NCORES):
    nc = bacc.Bacc("TRN2", target_bir_lowering=False, debug=False,
                   num_devices=num_devices)

    def din(name, shape, dt):
        return nc.dram_tensor(name, list(shape), dt, kind="ExternalInput").ap()

    att_b = din("att_b", [H, C, C], FP)
    seq_b = din("seq_b", [C, D], FP)
    seq_half = din("seq_half", [128, 4, D], BF)
    att_idx = din("att_idx", [128, H * 2], I32)
    seq_idx = din("seq_idx", [128, 2], I32)
    rs_idx = din("rs_idx", [128, 7], I32)
    s_att = din("s_att", [2, 128, 32], BF)
    s_lse = din("s_lse", [2, 128, 32], FP)
    oh_h = din("oh_h", [32, P], BF)
    oh_t = din("oh_t", [32, P], BF)
    ohm_h = din("ohm_h", [32, NPAD], FP)
    ohm_t = din("ohm_t", [32, NPAD], FP)
    head_wb = din("head_wb", [2 * D, EMB], BF)
    tail_wb = din("tail_wb", [2 * D, EMB], BF)
    head_bb = din("head_bb", [1, EMB], BF)
    tail_bb = din("tail_bb", [1, EMB], BF)
    bil_wb = din("bil_wb", [NK, 128, 32, L], BF)
    bil_bc = din("bil_bc", [128, 1], FP)
    pos_m = din("pos_m", [NPAD, RELS], FP)
    neg_m = din("neg_m", [NPAD, RELS], FP)
    priors_row = din("priors_row", [1, 2 * RELS], FP)
    ident_f = din("ident_f", [128, 128], FP)
    ones_col_f = din("ones_col_f", [128, 1], FP)
    ones_row_b = din("ones_row_b", [1, 128], BF)

    logits_out = nc.dram_tensor("logits_out", [NPC, L], FP,
                                kind="ExternalOutput").ap()
    risk_out = nc.dram_tensor("risk_out", [1, 1], FP, kind="ExternalOutput").ap()

    rs_part = nc.dram_tensor("rs_part", [RSROWS, P], FP).ap()
    rs_full = nc.dram_tensor("rs_full", [RSROWS, P], FP).ap()
    q_part = nc.dram_tensor("q_part", [1, 4 * RELS], FP).ap()
    q_full = nc.dram_tensor("q_full", [1, 4 * RELS], FP, addr_space="Shared").ap()

    att_rows = att_b.rearrange("h c (t x) -> (h c t) x", t=2)      # [H*C*2, CH]
    rs_rows = rs_full.rearrange("d (t n) -> (d t) n", t=2)         # [1552, 300]

    with tile.TileContext(nc) as tc, ExitStack() as ctx:
      pool = ctx.enter_context(tc.tile_pool(name="sb", bufs=1))
      for _rep in range(reps):
        # ---------------- small persistent loads ----------------
        satt_sb = [pool.tile([128, 32], BF, tag=f"satt{c}", name=f"satt{c}")
                   for c in range(2)]
        slse_sb = [pool.tile([128, 32], FP, tag=f"slse{c}", name=f"slse{c}")
                   for c in range(2)]
        for c in range(2):
            nc.sync.dma_start(satt_sb[c][:], s_att[c])
            nc.sync.dma_start(slse_sb[c][:], s_lse[c])
        ohh_sb = pool.tile([32, P], BF, tag="ohh")
        oht_sb = pool.tile([32, P], BF, tag="oht")
        ohmh_sb = pool.tile([32, NPAD], FP, tag="ohmh")
        ohmt_sb = pool.tile([32, NPAD], FP, tag="ohmt")
        nc.sync.dma_start(ohh_sb[:], oh_h[:])
        nc.sync.dma_start(oht_sb[:], oh_t[:])
        nc.sync.dma_start(ohmh_sb[:], ohm_h[:])
        nc.sync.dma_start(ohmt_sb[:], ohm_t[:])
        aidx_sb = pool.tile([128, H * 2], I32, tag="aidx")
        sidx_sb = pool.tile([128, 2], I32, tag="sidx")
        ridx_sb = pool.tile([128, 7], I32, tag="ridx")
        nc.sync.dma_start(aidx_sb[:], att_idx[:])
        nc.sync.dma_start(sidx_sb[:], seq_idx[:])
        nc.sync.dma_start(ridx_sb[:], rs_idx[:])
        seqh_sb = pool.tile([128, 4, D], BF, tag="seqh")
        nc.sync.dma_start(seqh_sb[:], seq_half[:])
        hb_sb = pool.tile([1, EMB], BF, tag="hb")
        tb_sb = pool.tile([1, EMB], BF, tag="tb")
        nc.sync.dma_start(hb_sb[:], head_bb[:])
        nc.sync.dma_start(tb_sb[:], tail_bb[:])
        bilb_sb = pool.tile([128, 1], FP, tag="bilb")
        nc.sync.dma_start(bilb_sb[:], bil_bc[:])
        posm_sb = pool.tile([128, NT, RELS], FP, tag="posm")
        negm_sb = pool.tile([128, NT, RELS], FP, tag="negm")
        nc.sync.dma_start(posm_sb[:], pos_m.rearrange("(t p) r -> p t r", p=128))
        nc.sync.dma_start(negm_sb[:], neg_m.rearrange("(t p) r -> p t r", p=128))
        pri_sb = pool.tile([1, 2 * RELS], FP, tag="pri")
        nc.sync.dma_start(pri_sb[:], priors_row[:])
        id_sb = pool.tile([128, 128], FP, tag="ident")
        nc.sync.dma_start(id_sb[:], ident_f[:])
        onec_sb = pool.tile([128, 1], FP, tag="onec")
        nc.sync.dma_start(onec_sb[:], ones_col_f[:])
        oner_sb = pool.tile([1, 128], BF, tag="oner")
        nc.sync.dma_start(oner_sb[:], ones_row_b[:])
        bias_m1 = pool.tile([128, 1], FP, tag="biasm1")
        bias_p1 = pool.tile([128, 1], FP, tag="biasp1")
        nc.vector.memset(bias_m1[:], -1.0)
        nc.vector.memset(bias_p1[:], 1.0)

        # ---------------- ent_emb (logsumexp over mentions) ----------------
        entemb_sb = pool.tile([32, EMB], FP, tag="entemb")
        with tc.tile_pool(name="ps_lse", bufs=1, space="PSUM") as ps_lse, \
             tc.tile_pool(name="sb_lse", bufs=2) as sp_lse:
            lse_ps = ps_lse.tile([32, 2, 512], FP, tag="lse")
            expr_tiles = []
            for chunk in range(2):
                srows = sp_lse.tile([128, D], FP, tag="srows")
                nc.gpsimd.indirect_dma_start(
                    out=srows[:], out_offset=None, in_=seq_b[:],
                    in_offset=bass.IndirectOffsetOnAxis(
                        ap=sidx_sb[:, chunk:chunk + 1], axis=0))
                expr = sp_lse.tile([128, D], FP, tag="expr")
                nc.scalar.activation(expr[:], srows[:], AF.Exp)
                expr_tiles.append(expr)
            for chunk in range(2):
                for nh in range(2):
                    nc.tensor.matmul(
                        lse_ps[:, nh, :384], slse_sb[chunk][:],
                        expr_tiles[chunk][:, nh * 384:(nh + 1) * 384],
                        start=(chunk == 0), stop=(chunk == 1))
            nc.scalar.activation(entemb_sb[:30, 0:384], lse_ps[:30, 0, :384], AF.Ln)
            nc.scalar.activation(entemb_sb[:30, 384:768], lse_ps[:30, 1, :384], AF.Ln)

        # ---------------- ent_att per head ----------------
        entatt_sb = pool.tile([32, H, CH], BF, tag="entatt")
        with tc.tile_pool(name="ps_att", bufs=2, space="PSUM") as ps_att, \
             tc.tile_pool(name="sb_ea", bufs=3) as sp_ea:
            for h in range(H):
                rows_bf = []
                for chunk in range(2):
                    arows = sp_ea.tile([128, CH], FP, tag="arows")
                    nc.gpsimd.indirect_dma_start(
                        out=arows[:], out_offset=None, in_=att_rows,
                        in_offset=bass.IndirectOffsetOnAxis(
                            ap=aidx_sb[:, h * 2 + chunk:h * 2 + chunk + 1], axis=0))
                    abf = sp_ea.tile([128, CH], BF, tag="abf")
                    nc.vector.tensor_copy(abf[:], arows[:])
                    rows_bf.append(abf)
                ea_ps = ps_att.tile([32, CH], FP, tag="ea")
                for chunk in range(2):
                    nc.tensor.matmul(ea_ps[:], satt_sb[chunk][:], rows_bf[chunk][:],
                                     start=(chunk == 0), stop=(chunk == 1))
                nc.scalar.activation(entatt_sb[:, h, :], ea_ps[:], AF.Copy)

        # ---------------- pair products -> ht_raw ----------------
        ht_raw = pool.tile([128, 4, P], FP, tag="htraw")
        with tc.tile_pool(name="ps_pp", bufs=1, space="PSUM") as ps_pp, \
             tc.tile_pool(name="sb_pp", bufs=2) as sp_pp:
            for cc in range(4):
                for ph in range(2):
                    prod = sp_pp.tile([128, H, NPC], BF, tag="prod")
                    for hg in range(4):
                        hA = ps_pp.tile([128, 3, 512], FP, tag="hA", bufs=1)
                        tA = ps_pp.tile([128, 3, 512], FP, tag="tA", bufs=1)
                        for hi in range(3):
                            h = hg * 3 + hi
                            ea_h = entatt_sb[:30, h, cc * 128:(cc + 1) * 128]
                            nc.tensor.matmul(
                                hA[:, hi, :NPC], ea_h,
                                ohh_sb[:30, ph * NPC:(ph + 1) * NPC],
                                start=True, stop=True)
                            nc.tensor.matmul(
                                tA[:, hi, :NPC], ea_h,
                                oht_sb[:30, ph * NPC:(ph + 1) * NPC],
                                start=True, stop=True)
                        tAb = sp_pp.tile([128, 3, NPC], BF, tag="tAb")
                        hAb = sp_pp.tile([128, 3, NPC], BF, tag="hAb")
                        nc.scalar.activation(tAb[:], tA[:, :, :NPC], AF.Copy)
                        nc.scalar.activation(hAb[:], hA[:, :, :NPC], AF.Copy)
                        nc.vector.tensor_tensor(
                            prod[:, hg * 3:(hg + 1) * 3, :],
                            hAb[:], tAb[:], op=ALU.mult)
                    # tree-reduce over h (dense bf16 adds)
                    l1 = sp_pp.tile([128, 6, NPC], BF, tag="l1")
                    nc.vector.tensor_tensor(l1[:], prod[:, 0:6, :],
                                            prod[:, 6:12, :], op=ALU.add)
                    l2 = sp_pp.tile([128, 3, NPC], BF, tag="l2")
                    nc.vector.tensor_tensor(l2[:], l1[:, 0:3, :],
                                            l1[:, 3:6, :], op=ALU.add)
                    l3 = sp_pp.tile([128, 1, NPC], BF, tag="l3")
                    nc.vector.tensor_tensor(l3[:], l2[:, 0:1, :],
                                            l2[:, 1:2, :], op=ALU.add)
                    nc.vector.tensor_tensor(
                        ht_raw[:, cc, ph * NPC:(ph + 1) * NPC],
                        l3[:, 0, :], l2[:, 2, :], op=ALU.add)

        # ---------------- nsum + rsT partials -> DRAM ----------------
        with tc.tile_pool(name="ps_rs", bufs=2, space="PSUM") as ps_rs, \
             tc.tile_pool(name="sb_rs", bufs=2) as sp_rs:
            ns_ps = ps_rs.tile([1, 2, 512], FP, tag="nsum")
            for cc in range(4):
                for ph in range(2):
                    nc.tensor.matmul(ns_ps[:, ph, :NPC], onec_sb[:],
                                     ht_raw[:, cc, ph * NPC:(ph + 1) * NPC],
                                     start=(cc == 0), stop=(cc == 3))
            ns_sb = pool.tile([1, 2, NPC], FP, tag="nsumsb")
            nc.scalar.activation(ns_sb[:], ns_ps[:, :, :NPC], AF.Copy)
            nc.sync.dma_start(rs_part[D:D + 1, :],
                              ns_sb[:].rearrange("a t n -> a (t n)"))
            zpad = pool.tile([1, P], FP, tag="zpad")
            nc.vector.memset(zpad[:], 0.0)
            for zr in range(D + 1, RSROWS):
                nc.sync.dma_start(rs_part[zr:zr + 1, :], zpad[:])

            htb = pool.tile([128, 4, P], BF, tag="htb")
            nc.vector.tensor_copy(htb[:], ht_raw[:])
            for dc in range(6):
                rs_ps = ps_rs.tile([128, 2, 512], FP, tag="rsps")
                for cc in range(4):
                    for ph in range(2):
                        nc.tensor.matmul(
                            rs_ps[:, ph, :NPC],
                            seqh_sb[:, cc, dc * 128:(dc + 1) * 128],
                            htb[:, cc, ph * NPC:(ph + 1) * NPC],
                            start=(cc == 0), stop=(cc == 3))
                rs_sb = sp_rs.tile([128, 2, NPC], FP, tag="rssb")
                nc.scalar.activation(rs_sb[:], rs_ps[:, :, :NPC], AF.Copy)
                nc.sync.dma_start(rs_part[dc * 128:(dc + 1) * 128, :],
                                  rs_sb[:].rearrange("p t n -> p (t n)"))

        if no_collectives:
            nc.sync.dma_start(rs_full[:], rs_part[:])
        else:
            nc.gpsimd.collective_compute(
                "AllReduce", ALU.add, replica_groups=_PAIR_GROUPS,
                ins=[rs_part[:]], outs=[rs_full[:]])

        # ---------------- load reduced rs + nsum (core-dependent via gather) ----
        rsm = pool.tile([128, 6, NPAD], BF, tag="rsm")
        nc.vector.memset(rsm[:], 0.0)
        invr = pool.tile([1, NPC], FP, tag="invr")
        with tc.tile_pool(name="sb_rl", bufs=2) as sp_rl:
            for dc in range(6):
                rf = sp_rl.tile([128, NPC], FP, tag="rsf")
                nc.gpsimd.indirect_dma_start(
                    out=rf[:], out_offset=None, in_=rs_rows,
                    in_offset=bass.IndirectOffsetOnAxis(
                        ap=ridx_sb[:, dc:dc + 1], axis=0))
                nc.vector.tensor_copy(rsm[:, dc, :NPC], rf[:])
            nsf = sp_rl.tile([128, NPC], FP, tag="nsf")
            nc.gpsimd.indirect_dma_start(
                out=nsf[:], out_offset=None, in_=rs_rows,
                in_offset=bass.IndirectOffsetOnAxis(ap=ridx_sb[:, 6:7], axis=0))
            nc.scalar.activation(invr[:], nsf[0:1, :], AF.Copy, bias=EPS)
        nc.vector.reciprocal(invr[:], invr[:])
        invc = pool.tile([128, NT], FP, tag="invc")
        nc.vector.memset(invc[:], 0.0)
        with tc.tile_pool(name="ps_inv", bufs=2, space="PSUM") as ps_inv:
            for nt in range(NT):
                w = 128 if nt < 2 else NPC - 256
                tp = ps_inv.tile([128, 1], FP, tag="invtp")
                nc.tensor.transpose(tp[:w, :], invr[:, nt * 128:nt * 128 + w],
                                    id_sb[:1, :1])
                nc.scalar.activation(invc[:w, nt:nt + 1], tp[:w, :], AF.Copy)

        # ---------------- hs/ts one-hot gathers ----------------
        hsT = pool.tile([128, 6, NPAD], BF, tag="hsT")
        tsT = pool.tile([128, 6, NPAD], BF, tag="tsT")
        with tc.tile_pool(name="ps_g", bufs=2, space="PSUM") as ps_g:
            for dc in range(6):
                for (dst, ohm_sb) in ((hsT, ohmh_sb), (tsT, ohmt_sb)):
                    g_ps = ps_g.tile([128, NPAD], FP, tag="gps")
                    nc.tensor.matmul(g_ps[:],
                                     entemb_sb[:30, dc * 128:(dc + 1) * 128],
                                     ohm_sb[:30, :], start=True, stop=True)
                    nc.scalar.activation(dst[:, dc, :], g_ps[:], AF.Copy)

        # ---------------- FFN ----------------
        hz = pool.tile([128, NT, EMB], BF, tag="hz")
        tz = pool.tile([128, NT, EMB], BF, tag="tz")
        for (xT, wdram, bias_sb, out_sb, wtag) in (
            (hsT, head_wb, hb_sb, hz, "hw"),
            (tsT, tail_wb, tb_sb, tz, "tw"),
        ):
            with tc.tile_pool(name=f"w_{wtag}", bufs=1) as wpool, \
                 tc.tile_pool(name=f"sb_f_{wtag}", bufs=2) as sp_f, \
                 tc.tile_pool(name=f"ps_f_{wtag}", bufs=2, space="PSUM") as ps_f:
                wsb = wpool.tile([128, 12, EMB], BF, tag=wtag)
                nc.sync.dma_start(wsb[:],
                                  wdram.rearrange("(dc p) e -> p dc e", p=128))
                for nt in range(NT):
                    for ec in range(2):
                        esl = slice(ec * 384, (ec + 1) * 384)
                        psA = ps_f.tile([128, 384], FP, tag="psA")
                        psB = ps_f.tile([128, 384], FP, tag="psB")
                        nc.tensor.matmul(psA[:], oner_sb[:], bias_sb[:, esl],
                                         start=True, stop=False)
                        for dc in range(6):
                            nc.tensor.matmul(
                                psA[:], xT[:, dc, nt * 128:(nt + 1) * 128],
                                wsb[:, dc, esl], start=False, stop=(dc == 5))
                        for dc in range(6):
                            nc.tensor.matmul(
                                psB[:], rsm[:, dc, nt * 128:(nt + 1) * 128],
                                wsb[:, dc + 6, esl], start=(dc == 0), stop=(dc == 5))
                        t1 = sp_f.tile([128, 384], FP, tag="t1")
                        nc.vector.tensor_scalar(t1[:], psB[:], invc[:, nt:nt + 1],
                                                None, op0=ALU.mult)
                        t2 = sp_f.tile([128, 384], FP, tag="t2")
                        nc.vector.tensor_tensor(t2[:], psA[:], t1[:], op=ALU.add)
                        nc.scalar.activation(out_sb[:, nt, esl], t2[:], AF.Tanh)

        # ---------------- bilinear ----------------
        lgT_sb = pool.tile([L, NPAD], FP, tag="lgTsb")
        with tc.tile_pool(name="bl", bufs=3) as blpool, \
             tc.tile_pool(name="blt", bufs=2) as bltpool, \
             tc.tile_pool(name="bw", bufs=2) as bwpool, \
             tc.tile_pool(name="ps_bil", bufs=1, space="PSUM") as ps_bil:
            lgT_ps = ps_bil.tile([L, NPAD], FP, tag="lgT")
            for k in range(NK):
                wk = bwpool.tile([128, 32, L], BF, tag="bilw")
                nc.sync.dma_start(wk[:], bil_wb[k])
                blT = bltpool.tile([128, 32, NPAD], BF, tag="blT")
                for nt in range(NT):
                    bl = blpool.tile([128, BLK * BLK], BF, tag="blgen")
                    hz_k = hz[:, nt, k * BLK:(k + 1) * BLK]
                    tz_k = tz[:, nt, k * BLK:(k + 1) * BLK]
                    eng = nc.vector if k < 8 else nc.gpsimd
                    eng.tensor_tensor(
                        bl[:],
                        hz_k.unsqueeze(2).broadcast_to([128, BLK, BLK]),
                        tz_k.unsqueeze(1).broadcast_to([128, BLK, BLK]),
                        op=ALU.mult)
                    nc.sync.dma_start_transpose(
                        blT[:, :, nt * 128:(nt + 1) * 128], bl[:])
                for q in range(32):
                    nc.tensor.matmul(lgT_ps[:], wk[:, q, :], blT[:, q, :],
                                     start=(k == 0 and q == 0),
                                     stop=(k == NK - 1 and q == 31))
            nc.scalar.activation(lgT_sb[:], lgT_ps[:], AF.Identity,
                                 bias=bilb_sb[:L, :])

        # ---------------- logits out + risk partials ----------------
        lg_sb = pool.tile([128, NT, L], FP, tag="lgsb")
        with tc.tile_pool(name="ps_out", bufs=2, space="PSUM") as ps_out, \
             tc.tile_pool(name="sb_out", bufs=2) as sp_out:
            qp_ps = ps_out.tile([1, 3 * RELS], FP, tag="qps")
            qp_ps2 = ps_out.tile([1, RELS], FP, tag="qps2")
            for nt in range(NT):
                tp = ps_out.tile([128, L], FP, tag="lgtp")
                nc.tensor.transpose(tp[:], lgT_sb[:, nt * 128:(nt + 1) * 128],
                                    id_sb[:L, :L])
                nc.scalar.activation(lg_sb[:, nt, :], tp[:], AF.Copy)
                rows = 128 if nt < 2 else NPC - 256
                nc.sync.dma_start(logits_out[nt * 128:nt * 128 + rows, :],
                                  lg_sb[:rows, nt, :])
                score = sp_out.tile([128, RELS], FP, tag="score")
                nc.vector.tensor_tensor(
                    score[:], lg_sb[:, nt, 1:L],
                    lg_sb[:, nt, 0:1].broadcast_to([128, RELS]),
                    op=ALU.subtract)
                sqA = sp_out.tile([128, RELS], FP, tag="sqA")
                sqB = sp_out.tile([128, RELS], FP, tag="sqB")
                nc.scalar.activation(sqA[:], score[:], AF.Square, bias=bias_m1[:])
                nc.scalar.activation(sqB[:], score[:], AF.Square, bias=bias_p1[:])
                qa = sp_out.tile([128, 3, RELS], FP, tag="qa")
                nc.vector.tensor_tensor(qa[:, 0, :], posm_sb[:, nt, :], sqA[:],
                                        op=ALU.mult)
                nc.vector.tensor_tensor(qa[:, 1, :], posm_sb[:, nt, :], sqB[:],
                                        op=ALU.mult)
                nc.vector.tensor_tensor(qa[:, 2, :], negm_sb[:, nt, :], sqB[:],
                                        op=ALU.mult)
                nc.tensor.matmul(qp_ps[:], onec_sb[:], qa[:],
                                 start=(nt == 0), stop=(nt == NT - 1))
                nc.tensor.matmul(qp_ps2[:], onec_sb[:],
                                 posm_sb[:, nt, :],
                                 start=(nt == 0), stop=(nt == NT - 1))
            qp_sb = pool.tile([1, 4 * RELS], FP, tag="qpsb")
            nc.scalar.activation(qp_sb[:, 0:3 * RELS], qp_ps[:], AF.Copy)
            nc.scalar.activation(qp_sb[:, 3 * RELS:4 * RELS], qp_ps2[:], AF.Copy)
            nc.sync.dma_start(q_part[:], qp_sb[:])

        if no_collectives:
            nc.sync.dma_start(q_full[:], q_part[:])
        else:
            nc.gpsimd.collective_compute(
                "AllReduce", ALU.add, replica_groups=_ALL_GROUPS,
                ins=[q_part[:]], outs=[q_full[:]])

        # ---------------- final risk ----------------
        qf = pool.tile([1, 4 * RELS], FP, tag="qf")
        nc.sync.dma_start(qf[:], q_full[:])
        R = RELS
        Q1, Q2, Q3, Q4 = (qf[:, i * R:(i + 1) * R] for i in range(4))
        po = pri_sb[:, 0:R]
        pl = pri_sb[:, R:2 * R]
        t = pool.tile([1, 16 * R], FP, tag="scratch")

        def s(i):
            return t[:, i * R:(i + 1) * R]

        nc.vector.tensor_scalar(s(0), Q4, 1.0, None, op0=ALU.max)
        nc.vector.reciprocal(s(0), s(0))                                 # rpos
        nc.vector.tensor_scalar(s(1), Q4, -1.0, float(B * P), op0=ALU.mult,
                                op1=ALU.add)
        nc.vector.tensor_scalar(s(1), s(1), 1.0, None, op0=ALU.max)
        nc.vector.reciprocal(s(1), s(1))                                 # rneg
        nc.vector.tensor_tensor(s(2), Q1, s(0), op=ALU.mult)             # spp
        nc.vector.tensor_tensor(s(3), Q2, s(0), op=ALU.mult)             # spn
        nc.vector.tensor_tensor(s(4), Q3, s(1), op=ALU.mult)             # sneg
        nc.vector.tensor_scalar(s(5), po, -1.0, 1.0, op0=ALU.mult, op1=ALU.add)
        nc.vector.reciprocal(s(6), po)
        nc.vector.tensor_tensor(s(6), s(5), s(6), op=ALU.mult)
        nc.scalar.activation(s(6), s(6), AF.Sqrt)                        # weight
        nc.vector.tensor_scalar(s(7), pl, -1.0, 1.0, op0=ALU.mult, op1=ALU.add)
        nc.vector.reciprocal(s(7), s(7))
        nc.vector.tensor_tensor(s(8), po, pl, op=ALU.subtract)
        nc.vector.tensor_tensor(s(8), s(8), s(7), op=ALU.mult)           # pu
        nc.vector.tensor_scalar(s(9), s(8), -1.0, 1.0, op0=ALU.mult, op1=ALU.add)
        nc.vector.reciprocal(s(9), s(9))                                 # 1/(1-pu)
        nc.vector.tensor_tensor(s(10), s(5), s(9), op=ALU.mult)
        nc.vector.tensor_tensor(s(10), s(10), s(4), op=ALU.mult)         # termA
        nc.vector.tensor_tensor(s(11), s(8), po, op=ALU.mult)
        nc.vector.tensor_tensor(s(11), s(8), s(11), op=ALU.subtract)
        nc.vector.tensor_tensor(s(11), s(11), s(9), op=ALU.mult)
        nc.vector.tensor_tensor(s(11), s(11), s(3), op=ALU.mult)         # termB
        nc.vector.tensor_tensor(s(12), s(10), s(11), op=ALU.subtract)    # risk1
        nc.vector.tensor_tensor(s(13), po, s(2), op=ALU.mult)
        nc.vector.tensor_tensor(s(13), s(13), s(6), op=ALU.mult)         # risk2
        nc.vector.tensor_scalar(s(14), s(12), 0.0, None, op0=ALU.is_ge)
        nc.vector.tensor_tensor(s(15), s(12), s(13), op=ALU.add)
        nc.vector.tensor_tensor(s(15), s(15), s(14), op=ALU.mult)
        nc.vector.tensor_scalar(s(12), s(12), -1.0, None, op0=ALU.mult)
        nc.scalar.activation(s(12), s(12), AF.Relu)
        nc.vector.tensor_tensor(s(15), s(15), s(12), op=ALU.add)
        nc.vector.tensor_scalar(s(15), s(15), 0.25, None, op0=ALU.mult)
        risk_sb = pool.tile([1, 1], FP, tag="risk")
        nc.vector.tensor_reduce(risk_sb[:], s(15), axis=AX.X, op=ALU.add)
        nc.sync.dma_start(risk_out[:], risk_sb[:])

    nc.compile()
    return nc


def host_shard(inputs):
    import ml_dtypes

    seq = np.asarray(inputs["sequence_output"], np.float32)
    att = np.asarray(inputs["attention"], np.float32)
    mid = np.asarray(inputs["mention_idx"], np.int32)
    mm = np.asarray(inputs["mention_mask"], np.float32)
    hts = np.asarray(inputs["hts"], np.int32)
    labels = np.asarray(inputs["labels"], np.int32)
    priors_l = np.asarray(inputs["priors_l"], np.float32)
    priors_o = np.asarray(inputs["priors_o"], np.float32)
    head_W = np.asarray(inputs["head_W"], np.float32)
    head_b = np.asarray(inputs["head_b"], np.float32)
    tail_W = np.asarray(inputs["tail_W"], np.float32)
    tail_b = np.asarray(inputs["tail_b"], np.float32)
    bil_W = np.asarray(inputs["bil_W"], np.float32)
    bil_b = np.asarray(inputs["bil_b"], np.float32)

    def to_bf(x):
        return np.ascontiguousarray(np.asarray(x, dtype=ml_dtypes.bfloat16))

    idx = mid + OFFSET
    cnt = mm.sum(2)

    bilw_r = to_bf(bil_W.reshape(NK, 32, 128, L).transpose(0, 2, 1, 3))
    bil_bc = np.zeros((128, 1), np.float32)
    bil_bc[:L, 0] = bil_b
    priors_row = np.concatenate([priors_o, priors_l])[None, :].astype(np.float32)
    ident = np.eye(128, dtype=np.float32)
    ones_col = np.ones((128, 1), np.float32)
    ones_row = to_bf(np.ones((1, 128), np.float32))

    in_maps = []
    for core in range(NCORES):
        b, i = core // 2, core % 2
        rows = idx[b].reshape(-1)
        aidx = np.zeros((128, H * 2), np.int32)
        for h in range(H):
            fr = (h * C + rows) * 2 + i
            fr = np.concatenate([fr, np.zeros(2 * 128 - EM, np.int32)])
            aidx[:, h * 2 + 0] = fr[:128]
            aidx[:, h * 2 + 1] = fr[128:]
        sidx = np.zeros((128, 2), np.int32)
        sr = np.concatenate([rows, np.zeros(2 * 128 - EM, np.int32)])
        sidx[:, 0] = sr[:128]
        sidx[:, 1] = sr[128:]
        ridx = np.zeros((128, 7), np.int32)
        for dc in range(6):
            ridx[:, dc] = (dc * 128 + np.arange(128)) * 2 + i
        ridx[:, 6] = D * 2 + i

        s_att = np.zeros((2, 128, 32), np.float32)
        s_lse = np.zeros((2, 128, 32), np.float32)
        w_att = (mm[b] / cnt[b][:, None]).reshape(-1)
        w_lse = mm[b].reshape(-1)
        ee = np.repeat(np.arange(E), M)
        for r in range(EM):
            s_att[r // 128, r % 128, ee[r]] = w_att[r]
            s_lse[r // 128, r % 128, ee[r]] = w_lse[r]

        h_i, t_i = hts[b, :, 0], hts[b, :, 1]
        oh_h = np.zeros((32, P), np.float32); oh_h[h_i, np.arange(P)] = 1.0
        oh_t = np.zeros((32, P), np.float32); oh_t[t_i, np.arange(P)] = 1.0
        ohm_h = np.zeros((32, NPAD), np.float32)
        ohm_t = np.zeros((32, NPAD), np.float32)
        sl = slice(i * NPC, (i + 1) * NPC)
        ohm_h[:, :NPC] = oh_h[:, sl]
        ohm_t[:, :NPC] = oh_t[:, sl]

        seq_half = to_bf(seq[b, i * CH:(i + 1) * CH, :].reshape(4, 128, D)
                         .transpose(1, 0, 2))

        nglob = b * P + i * NPC + np.arange(NPC)
        lab = labels[nglob].astype(np.float32)
        pos = np.zeros((NPAD, RELS), np.float32)
        neg = np.zeros((NPAD, RELS), np.float32)
        pos[:NPC] = (lab[:, 1:L] == 1.0)
        neg[:NPC] = 1.0 - pos[:NPC]

        in_maps.append({
            "att_b": att[b],
            "seq_b": seq[b],
            "seq_half": seq_half,
            "att_idx": aidx,
            "seq_idx": sidx,
            "rs_idx": ridx,
            "s_att": to_bf(s_att),
            "s_lse": s_lse,
            "oh_h": to_bf(oh_h),
            "oh_t": to_bf(oh_t),
            "ohm_h": ohm_h,
            "ohm_t": ohm_t,
            "head_wb": to_bf(head_W),
            "tail_wb": to_bf(tail_W),
            "head_bb": to_bf(head_b[None, :]),
            "tail_bb": to_bf(tail_b[None, :]),
            "bil_wb": bilw_r,
            "bil_bc": bil_bc,
            "pos_m": pos,
            "neg_m": neg,
            "priors_row": priors_row,
            "ident_f": ident,
            "ones_col_f": ones_col,
            "ones_row_b": ones_row,
        })
    return in_maps


_PROGRAM = None


def kernel(**inputs):
    global _PROGRAM, LAST_EXEC_NS
    from concourse.bass_utils import run_bass_kernel_spmd
    if _PROGRAM is None:
        _PROGRAM = build_program()
    in_maps = host_shard(inputs)
    import os as _os
    trace = bool(_os.environ.get("BASS_KERNEL_TRACE"))
    res = run_bass_kernel_spmd(_PROGRAM, in_maps, list(range(NCORES)),
                               trace=trace)
    LAST_EXEC_NS = res.exec_time_ns
    parts = [np.asarray(res.results[c]["logits_out"]) for c in range(NCORES)]
    logits = np.concatenate(parts, 0).astype(np.float32)
    risk = np.float32(np.asarray(res.results[0]["risk_out"]).reshape(()))
    return risk, logits
